# revision 18
# baseline (speedup 1.0000x reference)
"""3D Haar DWT (single level) on Trainium2, data-parallel over 8 NeuronCores.

Input  x: [2, 32, 32, 128, 128] f32  (B, C, D, H, W)
Output (LLL [2,32,16,64,64], H_all [2,224,16,64,64])  -- same pytree as the
reference: H_all = concat([LLH, LHL, LHH, HLL, HLH, HHL, HHH], axis=1).

Sharding: pure data parallel over the 64 (b, c) slices; core m owns the 8
contiguous slices [8m, 8m+8).  Per core the kernel computes all 8 subbands
of its [8, 32, 128, 128] block.

On-core layout: partitions = (n, k) where n = local slice, k = d-pair index
(8 * 16 = 128 partitions).  Free dims hold (d-parity, h, w), so all three
Haar butterflies are elementwise ops along the free axis.

This walrus build gives most instruction encodings a SINGLE sync-wait slot
(2D DMAs get two), so the dataflow is arranged as a linear chain with at
most one cross-engine dependency per instruction:
  - DVE runs all three butterfly levels (plain tensor_add/sub),
  - ACT scales lD/hD by s^3 in place between the D and H levels,
  - loads ride the SP HWDGE queue, stores the ACT HWDGE queue.
The one wait Tile emits that cannot fit -- the load's write-after-write
wait on the DMA lane of the load two generations earlier -- is provably
implied by the load's other wait (the X-tile readers saw that DMA finish),
and is stripped by _prune_redundant_dma_waits below.
"""

import numpy as np

from concourse import bass, mybir
from concourse.bass_utils import run_bass_kernel_spmd
from concourse.tile import TileContext

_S3 = 2.0**-1.5  # (1/sqrt(2))**3 -- the three Haar levels' combined scale

N_CORES = 8
F32 = mybir.dt.float32


def _prune_redundant_dma_waits(nc, verbose=False):
    """Drop DMAHW-semaphore waits from DMACopy instructions when they are
    transitively implied by the instruction's other waits.

    Tile's semaphore pass is per-proc minimal but not transitively minimal
    across procs: a reload of a tile slot waits both on the consumer engine
    (slot release) and on the DMA-lane semaphore of the slot's previous
    filler, even though the consumer's own data wait already implies the
    latter.  walrus' looped-DMA encoding has one sync-wait slot, so the
    redundant lane wait must go.

    Soundness: knowledge is tracked per proc (engine) in scheduled block
    order.  A proc learns (sem >= v) from its own waits, and importing
    through a wait on a single-producer engine semaphore merges the
    producer's knowledge snapshot at that increment (waits execute at the
    sequencer before the instruction, so program order carries knowledge).
    A DMAHW wait is dropped only if implied by that knowledge.
    """
    insts = []
    for b in nc.m.functions[0].blocks:
        insts.extend(b.instructions)

    # Identify single-producer pure-increment semaphores (per-engine sems).
    producers = {}  # sem id -> set of engines
    impure = set()  # sems with dec or register updates
    for i in insts:
        si = i.sync_info
        for u in (si.on_update or []) if si else []:
            if u.sync_type != "semaphore" or u.update_mode != "sem-inc" or u.update_reg is not None:
                impure.add(u.id)
            producers.setdefault(u.id, set()).add(i.engine)
    single = {s for s, e in producers.items() if len(e) == 1 and s not in impure}

    know = {}  # engine -> {sem id: max known value}
    snaps = {}  # sem id -> list of (cum_value, knowledge dict copy)
    cum = {}  # sem id -> cumulative inc
    n_dropped = 0
    sem_engine = {s: next(iter(producers[s])) for s in single}
    # Engines whose ops complete in program order (each DVE/ACT op drains its
    # pipeline), so a wait on the engine's own semaphore at a value already
    # produced by earlier instructions is implied by program order.  GpSimd
    # (Pool) runs ops across 8 Q7 cores and is excluded.
    inorder = {mybir.EngineType.DVE, mybir.EngineType.Activation}

    def lookup(s, v):
        """Producer knowledge snapshot at the first increment reaching v."""
        for c, k in snaps.get(s, []):
            if c >= v:
                return k
        return None

    for i in insts:
        si = i.sync_info
        if si is None:
            continue
        waits = list(si.on_wait or [])
        k_eng = know.setdefault(i.engine, {})

        ok_waits = [
            w for w in waits
            if w.sync_type == "semaphore" and w.wait_mode == "sem-ge-imm" and w.wait_reg is None
        ]
        # Local view: proc knowledge + imports through single-producer waits.
        k_local = dict(k_eng)
        for w in ok_waits:
            if w.id in single:
                imp = lookup(w.id, w.wait_value)
                if imp:
                    for s, v in imp.items():
                        if k_local.get(s, -1) < v:
                            k_local[s] = v
            if k_local.get(w.id, -1) < w.wait_value:
                k_local[w.id] = w.wait_value

        op = str(i.opcode)
        if len(waits) > 1 and op != "EventSemaphore":
            ok_ids = {id(w) for w in ok_waits}
            kept = []
            for w in waits:
                redundant = False
                if id(w) in ok_ids:
                    # Implied transitively through the instruction's other
                    # waits?  (Mutual implication is impossible: a snapshot
                    # only carries knowledge from causally-earlier events.)
                    if _known_without(k_eng, ok_waits, w, single, lookup) >= w.wait_value:
                        redundant = True
                    elif (
                        op not in ("DMACopy", "Drain")
                        and w.id in single
                        and sem_engine[w.id] == i.engine
                        and i.engine in inorder
                        and cum.get(w.id, 0) >= w.wait_value
                    ):
                        # Engine-executed op waiting on its own in-order
                        # engine's semaphore at an already-produced value:
                        # implied by program order.  (DMACopy excluded: the
                        # sequencer issues DMAs ahead of engine completion.)
                        redundant = True
                if redundant:
                    n_dropped += 1
                else:
                    kept.append(w)
            if len(kept) != len(waits):
                si.on_wait = kept
                i.sync_info = si
                if verbose:
                    print(f"pruned {i.name}: {[w.ant_name for w in waits]} -> "
                          f"{[w.ant_name for w in kept]}")

        # Proc knowledge advances by everything this instruction waited on
        # (kept or dropped -- dropped ones were implied anyway).
        for s, v in k_local.items():
            if k_eng.get(s, -1) < v:
                k_eng[s] = v

        for u in (si.on_update or []):
            if u.id in single:
                c = cum.get(u.id, 0) + u.update_value
                cum[u.id] = c
                snap = dict(k_eng)
                snap[u.id] = c
                snaps.setdefault(u.id, []).append((c, snap))

    return n_dropped


def _known_without(k_eng, ok_waits, w, single, lookup):
    """Max value of w's semaphore implied by proc knowledge plus imports
    through the OTHER single-producer waits of the same instruction."""
    best = k_eng.get(w.id, -1)
    for o in ok_waits:
        if o is w or o.id not in single:
            continue
        imp = lookup(o.id, o.wait_value)
        if imp and imp.get(w.id, -1) > best:
            best = imp.get(w.id, -1)
    return best


def build_dwt_nc(N=8, D=32, H=128, W=128, hc=16):
    """Bass program for one core: x [N, D, H, W] -> y [8, N, D/2, H/2, W/2].

    y's dim 0 is the subband in (d, h, w) binary order:
    LLL, LLH, LHL, LHH, HLL, HLH, HHL, HHH.
    `hc` = rows of H processed per pipeline iteration (must be even, divide H).
    """
    K = D // 2
    P = N * K
    assert P <= 128 and H % hc == 0 and hc % 2 == 0
    nc = bass.Bass()
    x = nc.dram_tensor("x", [N, D, H, W], F32, kind="ExternalInput")
    y = nc.dram_tensor("y", [8, N, K, H // 2, W // 2], F32, kind="ExternalOutput")

    # (n, k) merge into one partition dim; (j, i) merge into one contiguous
    # free run -- keeps every DMA access pattern within the 3-dim limit.
    xr = x[:].rearrange("n (k dp) h w -> (n k) dp h w", dp=2)
    # Partition-major destination view so one DMA stores all 8 subbands:
    # walk order (n k) then subband then (j i) matches the SBUF tile walk.
    yr = y[:].rearrange("s n k j i -> (n k) s (j i)")

    hw2 = (hc // 2) * (W // 2)  # output elements per partition per iteration

    last_Os = []
    with TileContext(nc) as tc:
        with tc.tile_pool(name="pool", bufs=2) as pool:
            for it in range(H // hc):
                h0 = it * hc
                X = pool.tile([P, 2, hc, W], F32, tag="X")
                # Loads go through gpsimd's SWDGE queue: its DMASW lane
                # semaphores are used by nothing else here, so loads get no
                # lane-serialization pre-wait and fit the single wait slot.
                nc.gpsimd.dma_start(out=X[:], in_=xr[:, :, h0 : h0 + hc, :])

                lD = pool.tile([P, hc, W], F32, tag="lD")
                hD = pool.tile([P, hc, W], F32, tag="hD")
                nc.vector.tensor_add(out=lD[:], in0=X[:, 0], in1=X[:, 1])
                nc.vector.tensor_sub(out=hD[:], in0=X[:, 0], in1=X[:, 1])

                # Fold the overall s^3 scale in here (in place, on ACT):
                # every later level stays plain add/sub.
                nc.scalar.mul(lD[:], lD[:], _S3)
                nc.scalar.mul(hD[:], hD[:], _S3)

                # All 8 subbands go into one packed tile so the iteration
                # needs a single store DMA (one DMA lane per iteration means
                # no lane-serialization pre-waits anywhere).
                O = pool.tile([P, 8, hc // 2, W // 2], F32, tag="O")
                last_Os = (last_Os + [O])[-2:]
                for q, src in enumerate((lD, lD, hD, hD)):
                    hh = pool.tile([P, hc // 2, W], F32, tag=f"hh{q}")
                    ttop = nc.vector.tensor_add if q % 2 == 0 else nc.vector.tensor_sub
                    ttop(out=hh[:], in0=src[:, 0:hc:2, :], in1=src[:, 1:hc:2, :])

                    nc.vector.tensor_add(out=O[:, 2 * q], in0=hh[:, :, 0:W:2], in1=hh[:, :, 1:W:2])
                    nc.vector.tensor_sub(out=O[:, 2 * q + 1], in0=hh[:, :, 0:W:2], in1=hh[:, :, 1:W:2])

                ji0 = it * hw2
                nc.scalar.dma_start(
                    out=yr[:, :, ji0 : ji0 + hw2],
                    in_=O[:].rearrange("p s j i -> p s (j i)"),
                )

            # Overwrite one element of the last two O tiles after their
            # stores: the write-after-read waits absorb those stores'
            # DMA-lane semaphores into DVE, leaving the kernel-tail Drain
            # with only the DVE semaphore (it has one usable sync-wait slot).
            for Ot in last_Os:
                nc.vector.tensor_copy(out=Ot[0:1, 0, 0:1, 0:1], in_=lD[0:1, 0:1, 0:1])

    _prune_redundant_dma_waits(nc)
    return nc


_NC_CACHE = {}


def _get_nc(hc=16):
    if hc not in _NC_CACHE:
        _NC_CACHE[hc] = build_dwt_nc(hc=hc)
    return _NC_CACHE[hc]


def _execute(x, trace=False, hc=16, **spmd_kwargs):
    """Run the SPMD kernel on the full input; returns ((LLL, H_all), results)."""
    x = np.asarray(x, dtype=np.float32)
    assert x.shape == (2, 32, 32, 128, 128), x.shape
    x64 = x.reshape(64, 32, 128, 128)
    in_maps = [{"x": x64[8 * m : 8 * m + 8]} for m in range(N_CORES)]

    nc = _get_nc(hc)
    res = run_bass_kernel_spmd(
        nc, in_maps, list(range(N_CORES)), trace=trace, **spmd_kwargs
    )

    # Per-core y: [8 sub, 8 n, 16, 64, 64]; flat (b,c) index = 8*m + n.
    arr = np.concatenate([res.results[m]["y"] for m in range(N_CORES)], axis=1)
    arr = arr.reshape(8, 2, 32, 16, 64, 64)
    LLL = np.ascontiguousarray(arr[0])
    H_all = np.ascontiguousarray(arr[1:].transpose(1, 0, 2, 3, 4, 5)).reshape(
        2, 224, 16, 64, 64
    )
    return (LLL, H_all), res


def kernel(x):
    return _execute(x)[0]
